# revision 5
# baseline (speedup 1.0000x reference)
"""Trainium2 Bass kernel for nn_Attention (B=4, Nq=Nk=1024, D=512, H=8).

Sharding: 8 cores = 4 batches x 2 head-groups (4 heads each).
Core c handles batch b = c // 2, heads [hg*4, hg*4+4) with hg = c % 2.

Per-core device program (all heavy math on device):
  qT = (x_q[b] @ w_q[:, hg])^T          [256, 1024]   (f32r matmuls)
  kT = (x[b]  @ w_k[:, hg])^T           [256, 1024]
  v  =  x[b]  @ w_v[:, hg]              [1024, 256]   (stored bf16)
  per head h (4):
    A: logits[q,k] = qT_h^T kT_h; exp (ACT, scale=0.125 fused, accum row
       sums); normalize in-place (gpsimd, per-partition recip); DMA -> attn
    B: logitsT[k,q] = kT_h^T qT_h; expT = exp (ACT, bf16)
    AV: out_u[q,64] = sum_kj expT_kj^T v_kj (bf16 matmuls);
        scale by recip (DVE, fused into PSUM->SBUF copy)
  transpose out [1024,256] -> outT (PE transposes), proj = outT^T @ w_p
  partial out -> DRAM (host adds the two head-group partials + bias).

Host side: slice/transpose inputs (numpy), run SPMD on 8 cores, concat
attn shards, add the two per-batch partials + b_proj.
"""

import sys

import numpy as np

for _p in ("/opt/trn_rl_repo",):
    if _p not in sys.path:
        sys.path.insert(0, _p)

# Problem constants (hardcoded per contest rules).
B, NQ, NK = 4, 1024, 1024
D = 512          # DIM_Q = DIM_K = OUT_DIM
H = 8
HD = 64          # head dim
SCALE = HD ** -0.5
HPC = 4          # heads per core
DH = HPC * HD    # 256: per-core slice of head dims
P = 128
KO = D // P      # 4 contraction chunks for the input projections
NCORES = 8

_NC_CACHE = {}


def build_nc():
    """Build the (single) SPMD Bass program. Same program on all 8 cores."""
    from contextlib import ExitStack

    import concourse.mybir as mybir
    import concourse.tile as tile
    from concourse import bacc
    from concourse.masks import make_identity

    FP = mybir.dt.float32
    BF = mybir.dt.bfloat16
    FR = mybir.dt.float32r
    Exp = mybir.ActivationFunctionType.Exp

    nc = bacc.Bacc("TRN2")
    xqT = nc.declare_dram_parameter("xqT", [D, NQ], FR, isOutput=False)
    xT = nc.declare_dram_parameter("xT", [D, NK], FR, isOutput=False)
    wq = nc.declare_dram_parameter("wq", [D, DH], FR, isOutput=False)
    wk = nc.declare_dram_parameter("wk", [D, DH], FR, isOutput=False)
    wv = nc.declare_dram_parameter("wv", [D, DH], FR, isOutput=False)
    wp = nc.declare_dram_parameter("wp", [DH, D], FR, isOutput=False)
    attn_o = nc.declare_dram_parameter("attn_o", [HPC, NQ, NK], FP, isOutput=True)
    out_o = nc.declare_dram_parameter("out_o", [NQ, D], FP, isOutput=True)

    with tile.TileContext(nc) as tc:
        with ExitStack() as ctx:
            consts = ctx.enter_context(tc.tile_pool(name="consts", bufs=1))
            persist = ctx.enter_context(tc.tile_pool(name="persist", bufs=1))
            expT_pool = ctx.enter_context(tc.tile_pool(name="expT", bufs=2))
            exA_pool = ctx.enter_context(tc.tile_pool(name="exA", bufs=10))
            outp = ctx.enter_context(tc.tile_pool(name="outp", bufs=2))
            ps_mm = ctx.enter_context(tc.tile_pool(name="ps_mm", bufs=2, space="PSUM"))
            ps_lg = ctx.enter_context(tc.tile_pool(name="ps_lg", bufs=2, space="PSUM"))
            ps_av = ctx.enter_context(tc.tile_pool(name="ps_av", bufs=2, space="PSUM"))

            # ---- load inputs ----
            xqT_sb = persist.tile([P, KO, NQ], FR)
            nc.sync.dma_start(xqT_sb[:], xqT[:].rearrange("(ko p) q -> p ko q", p=P))
            xT_sb = persist.tile([P, KO, NK], FR)
            nc.sync.dma_start(xT_sb[:], xT[:].rearrange("(ko p) q -> p ko q", p=P))
            wq_sb = persist.tile([P, KO, DH], FR)
            nc.sync.dma_start(wq_sb[:], wq[:].rearrange("(ko p) m -> p ko m", p=P))
            wk_sb = persist.tile([P, KO, DH], FR)
            nc.sync.dma_start(wk_sb[:], wk[:].rearrange("(ko p) m -> p ko m", p=P))
            wv_sb = persist.tile([P, KO, DH], FR)
            nc.sync.dma_start(wv_sb[:], wv[:].rearrange("(ko p) m -> p ko m", p=P))
            wp_sb = persist.tile([P, 2, D], FR)
            nc.sync.dma_start(wp_sb[:], wp[:].rearrange("(j p) n -> p j n", p=P))
            ident = consts.tile([P, P], FP)
            make_identity(nc, ident[:])

            # ---- projections: qT, kT  ([dh-part(2x128), seq]) ----
            qT_sb = persist.tile([P, 2, NQ], FR)
            kT_sb = persist.tile([P, 2, NK], FR)
            for w_sb, x_sb, dst in ((wq_sb, xqT_sb, qT_sb), (wk_sb, xT_sb, kT_sb)):
                for j in range(2):
                    for qc in range(2):
                        ps = ps_mm.tile([P, 512], FP, tag="mm")
                        for ko in range(KO):
                            nc.tensor.matmul(
                                ps[:],
                                w_sb[:, ko, j * P:(j + 1) * P],
                                x_sb[:, ko, qc * 512:(qc + 1) * 512],
                                start=(ko == 0),
                                stop=(ko == KO - 1),
                            )
                        nc.vector.tensor_copy(dst[:, j, qc * 512:(qc + 1) * 512], ps[:])

            # ---- v in natural layout [k-part, kj, head, hd], bf16 ----
            v_sb = persist.tile([P, 8, HPC, HD], BF)
            for kj in range(8):
                ps = ps_mm.tile([P, DH], FP, tag="mm")
                for ko in range(KO):
                    nc.tensor.matmul(
                        ps[:],
                        xT_sb[:, ko, kj * P:(kj + 1) * P],
                        wv_sb[:, ko, :],
                        start=(ko == 0),
                        stop=(ko == KO - 1),
                    )
                nc.vector.tensor_copy(
                    v_sb[:, kj].rearrange("p h d -> p (h d)"), ps[:]
                )

            # ---- per-head attention ----
            sums = consts.tile([P, HPC * 8], FP)   # row sums per (head, q-tile)
            rec = consts.tile([P, HPC * 8], FP)    # reciprocals
            out_n = persist.tile([P, 8, DH], FP)   # normalized attn @ v, [q, dh]

            for h in range(HPC):
                j, p0 = h // 2, (h % 2) * 64
                qT_h = qT_sb[p0:p0 + 64, j]        # [64, NQ]
                kT_h = kT_sb[p0:p0 + 64, j]        # [64, NK]

                # --- A: logits [q, k] -> exp (+ row sums) ---
                exa_tiles = []
                for mi in range(8):
                    ps = ps_lg.tile([P, NK], FP, tag="lg")
                    for kc in range(2):
                        nc.tensor.matmul(
                            ps[:, kc * 512:(kc + 1) * 512],
                            qT_h[:, mi * P:(mi + 1) * P],
                            kT_h[:, kc * 512:(kc + 1) * 512],
                            start=True,
                            stop=True,
                        )
                    exa = exA_pool.tile([P, NK], FP, tag="exA")
                    si = h * 8 + mi
                    nc.scalar.activation(
                        exa[:], ps[:], Exp, scale=SCALE,
                        accum_out=sums[:, si:si + 1],
                    )
                    exa_tiles.append(exa)

                nc.vector.reciprocal(rec[:, h * 8:(h + 1) * 8], sums[:, h * 8:(h + 1) * 8])

                # --- B: logitsT [k, q] -> expT (bf16) ---
                expT = expT_pool.tile([P, 8, NQ], BF, tag="expT")
                for kj in range(8):
                    ps = ps_lg.tile([P, NQ], FP, tag="lg")
                    for qc in range(2):
                        nc.tensor.matmul(
                            ps[:, qc * 512:(qc + 1) * 512],
                            kT_h[:, kj * P:(kj + 1) * P],
                            qT_h[:, qc * 512:(qc + 1) * 512],
                            start=True,
                            stop=True,
                        )
                    nc.scalar.activation(expT[:, kj], ps[:], Exp, scale=SCALE)

                # --- normalize attn in place + store ---
                for mi in range(8):
                    si = h * 8 + mi
                    nc.gpsimd.tensor_scalar_mul(
                        exa_tiles[mi][:], exa_tiles[mi][:], rec[:, si:si + 1]
                    )
                    nc.sync.dma_start(
                        attn_o[h, mi * P:(mi + 1) * P, :], exa_tiles[mi][:]
                    )

                # --- AV: out_u[q, 64] = sum_kj expT_kj^T v_kj; scale by rec ---
                for mi in range(8):
                    psv = ps_av.tile([P, HD], FP, tag="av")
                    for kj in range(8):
                        nc.tensor.matmul(
                            psv[:],
                            expT[:, kj, mi * P:(mi + 1) * P],
                            v_sb[:, kj, h],
                            start=(kj == 0),
                            stop=(kj == 7),
                        )
                    si = h * 8 + mi
                    nc.vector.tensor_scalar_mul(
                        out_n[:, mi, h * HD:(h + 1) * HD], psv[:], rec[:, si:si + 1]
                    )

            # ---- transpose out [q, dh] -> outT [dh, q] ----
            outT = persist.tile([P, 2, NQ], FR)
            for mi in range(8):
                for j in range(2):
                    pst = ps_av.tile([P, P], FP, tag="av")
                    nc.tensor.transpose(pst[:], out_n[:, mi, j * P:(j + 1) * P], ident[:])
                    nc.vector.tensor_copy(outT[:, j, mi * P:(mi + 1) * P], pst[:])

            # ---- output projection ----
            for mi in range(8):
                ps = ps_mm.tile([P, D], FP, tag="mm")
                for j in range(2):
                    nc.tensor.matmul(
                        ps[:],
                        outT[:, j, mi * P:(mi + 1) * P],
                        wp_sb[:, j, :],
                        start=(j == 0),
                        stop=(j == 1),
                    )
                of = outp.tile([P, D], FP, tag="of")
                nc.vector.tensor_copy(of[:], ps[:])
                nc.sync.dma_start(out_o[mi * P:(mi + 1) * P, :], of[:])

    nc.compile()
    return nc


def get_nc():
    if "nc" not in _NC_CACHE:
        _NC_CACHE["nc"] = build_nc()
    return _NC_CACHE["nc"]


def make_in_maps(x, x_q, w_q, w_kv):
    """Shard full inputs into 8 per-core input maps (host-side numpy)."""
    x = np.asarray(x, dtype=np.float32)
    x_q = np.asarray(x_q, dtype=np.float32)
    w_q = np.asarray(w_q, dtype=np.float32)
    w_kv = np.asarray(w_kv, dtype=np.float32)
    in_maps = []
    for c in range(NCORES):
        b, hg = c // 2, c % 2
        sl = slice(hg * DH, (hg + 1) * DH)
        in_maps.append({
            "xqT": np.ascontiguousarray(x_q[b].T),
            "xT": np.ascontiguousarray(x[b].T),
            "wq": np.ascontiguousarray(w_q[:, sl]),
            "wk": np.ascontiguousarray(w_kv[:, sl]),
            "wv": np.ascontiguousarray(w_kv[:, D + hg * DH:D + (hg + 1) * DH]),
        })
    return in_maps


def make_in_maps_full(x, x_q, w_q, w_kv, w_proj):
    w_proj = np.asarray(w_proj, dtype=np.float32)
    in_maps = make_in_maps(x, x_q, w_q, w_kv)
    for c in range(NCORES):
        hg = c % 2
        sl = slice(hg * DH, (hg + 1) * DH)
        in_maps[c]["wp"] = np.ascontiguousarray(w_proj[sl, :])
    return in_maps


def unshard(results, b_proj):
    b_proj = np.asarray(b_proj, dtype=np.float32)
    attn = np.empty((B, H, NQ, NK), dtype=np.float32)
    out = np.empty((B, NQ, D), dtype=np.float32)
    for c in range(NCORES):
        b, hg = c // 2, c % 2
        attn[b, hg * HPC:(hg + 1) * HPC] = results[c]["attn_o"]
    for b in range(B):
        out[b] = results[2 * b]["out_o"] + results[2 * b + 1]["out_o"] + b_proj[None, :]
    return out, attn


def kernel(x, x_q, w_q, w_kv, w_proj, b_proj):
    from concourse.bass_utils import run_bass_kernel_spmd

    nc = get_nc()
    in_maps = make_in_maps_full(x, x_q, w_q, w_kv, w_proj)
    res = run_bass_kernel_spmd(nc, in_maps, list(range(NCORES))).results
    return unshard(res, b_proj)


# revision 6
# speedup vs baseline: 3.4860x; 3.4860x over previous
"""Trainium2 Bass kernel for nn_Attention (B=4, Nq=Nk=1024, D=512, H=8).

Sharding: 8 cores = 4 batches x 2 head-groups (4 heads each).
Core c handles batch b = c // 2, heads [hg*4, hg*4+4) with hg = c % 2.

Per-core device program (all heavy math on device):
  qT = (x_q[b] @ w_q[:, hg])^T          [256, 1024]   (f32r matmuls)
  kT = (x[b]  @ w_k[:, hg])^T           [256, 1024]
  v  =  x[b]  @ w_v[:, hg]              [1024, 256]   (stored bf16)
  per head h (4):
    A: logits[q,k] = qT_h^T kT_h; exp (ACT, scale=0.125 fused, accum row
       sums); normalize in-place (gpsimd, per-partition recip); DMA -> attn
    B: logitsT[k,q] = kT_h^T qT_h; expT = exp (ACT, bf16)
    AV: out_u[q,64] = sum_kj expT_kj^T v_kj (bf16 matmuls);
        scale by recip (DVE, fused into PSUM->SBUF copy)
  transpose out [1024,256] -> outT (PE transposes), proj = outT^T @ w_p
  partial out -> DRAM (host adds the two head-group partials + bias).

Host side: slice/transpose inputs (numpy), run SPMD on 8 cores, concat
attn shards, add the two per-batch partials + b_proj.
"""

import sys

import numpy as np

for _p in ("/opt/trn_rl_repo",):
    if _p not in sys.path:
        sys.path.insert(0, _p)

# Problem constants (hardcoded per contest rules).
B, NQ, NK = 4, 1024, 1024
D = 512          # DIM_Q = DIM_K = OUT_DIM
H = 8
HD = 64          # head dim
SCALE = HD ** -0.5
HPC = 4          # heads per core
DH = HPC * HD    # 256: per-core slice of head dims
P = 128
KO = D // P      # 4 contraction chunks for the input projections
NCORES = 8

_NC_CACHE = {}


def build_nc():
    """Build the (single) SPMD Bass program. Same program on all 8 cores."""
    from contextlib import ExitStack

    import concourse.mybir as mybir
    import concourse.tile as tile
    from concourse import bacc
    from concourse.masks import make_identity

    FP = mybir.dt.float32
    BF = mybir.dt.bfloat16
    FR = mybir.dt.float32r
    Exp = mybir.ActivationFunctionType.Exp

    nc = bacc.Bacc("TRN2")
    xqT = nc.declare_dram_parameter("xqT", [D, NQ], FR, isOutput=False)
    xT = nc.declare_dram_parameter("xT", [D, NK], FR, isOutput=False)
    wq = nc.declare_dram_parameter("wq", [D, DH], FR, isOutput=False)
    wk = nc.declare_dram_parameter("wk", [D, DH], FR, isOutput=False)
    wv = nc.declare_dram_parameter("wv", [D, DH], FR, isOutput=False)
    wp = nc.declare_dram_parameter("wp", [DH, D], FR, isOutput=False)
    attn_o = nc.declare_dram_parameter("attn_o", [HPC, NQ, NK], FP, isOutput=True)
    out_o = nc.declare_dram_parameter("out_o", [NQ, D], FP, isOutput=True)

    with tile.TileContext(nc) as tc:
        with ExitStack() as ctx:
            consts = ctx.enter_context(tc.tile_pool(name="consts", bufs=1))
            persist = ctx.enter_context(tc.tile_pool(name="persist", bufs=1))
            expT_pool = ctx.enter_context(tc.tile_pool(name="expT", bufs=2))
            exA_pool = ctx.enter_context(tc.tile_pool(name="exA", bufs=10))
            outp = ctx.enter_context(tc.tile_pool(name="outp", bufs=2))
            ps_mm = ctx.enter_context(tc.tile_pool(name="ps_mm", bufs=2, space="PSUM"))
            ps_lg = ctx.enter_context(tc.tile_pool(name="ps_lg", bufs=2, space="PSUM"))
            ps_av = ctx.enter_context(tc.tile_pool(name="ps_av", bufs=2, space="PSUM"))

            # ---- load inputs ----
            xqT_sb = persist.tile([P, KO, NQ], FR)
            nc.sync.dma_start(xqT_sb[:], xqT[:].rearrange("(ko p) q -> p ko q", p=P))
            xT_sb = persist.tile([P, KO, NK], FR)
            nc.sync.dma_start(xT_sb[:], xT[:].rearrange("(ko p) q -> p ko q", p=P))
            wq_sb = persist.tile([P, KO, DH], FR)
            nc.sync.dma_start(wq_sb[:], wq[:].rearrange("(ko p) m -> p ko m", p=P))
            wk_sb = persist.tile([P, KO, DH], FR)
            nc.sync.dma_start(wk_sb[:], wk[:].rearrange("(ko p) m -> p ko m", p=P))
            wv_sb = persist.tile([P, KO, DH], FR)
            nc.sync.dma_start(wv_sb[:], wv[:].rearrange("(ko p) m -> p ko m", p=P))
            wp_sb = persist.tile([P, 2, D], FR)
            nc.sync.dma_start(wp_sb[:], wp[:].rearrange("(j p) n -> p j n", p=P))
            ident = consts.tile([P, P], FP)
            make_identity(nc, ident[:])

            # ---- projections: qT, kT  ([dh-part(2x128), seq]) ----
            qT_sb = persist.tile([P, 2, NQ], FR)
            kT_sb = persist.tile([P, 2, NK], FR)
            for w_sb, x_sb, dst in ((wq_sb, xqT_sb, qT_sb), (wk_sb, xT_sb, kT_sb)):
                for j in range(2):
                    for qc in range(2):
                        ps = ps_mm.tile([P, 512], FP, tag="mm")
                        for ko in range(KO):
                            nc.tensor.matmul(
                                ps[:],
                                w_sb[:, ko, j * P:(j + 1) * P],
                                x_sb[:, ko, qc * 512:(qc + 1) * 512],
                                start=(ko == 0),
                                stop=(ko == KO - 1),
                            )
                        nc.vector.tensor_copy(dst[:, j, qc * 512:(qc + 1) * 512], ps[:])

            # ---- v in natural layout [k-part, kj, head, hd], bf16 ----
            v_sb = persist.tile([P, 8, HPC, HD], BF)
            for kj in range(8):
                ps = ps_mm.tile([P, DH], FP, tag="mm")
                for ko in range(KO):
                    nc.tensor.matmul(
                        ps[:],
                        xT_sb[:, ko, kj * P:(kj + 1) * P],
                        wv_sb[:, ko, :],
                        start=(ko == 0),
                        stop=(ko == KO - 1),
                    )
                nc.vector.tensor_copy(
                    v_sb[:, kj].rearrange("p h d -> p (h d)"), ps[:]
                )

            # ---- per-head attention ----
            sums = consts.tile([P, HPC * 8], FP)   # row sums per (head, q-tile)
            rec = consts.tile([P, HPC * 8], FP)    # reciprocals
            out_n = persist.tile([P, 8, DH], FP)   # normalized attn @ v, [q, dh]

            for h in range(HPC):
                j, p0 = h // 2, (h % 2) * 64
                qT_h = qT_sb[p0:p0 + 64, j]        # [64, NQ]
                kT_h = kT_sb[p0:p0 + 64, j]        # [64, NK]

                # --- A: logits [q, k] -> exp (+ row sums) ---
                exa_tiles = []
                for mi in range(8):
                    ps = ps_lg.tile([P, NK], FP, tag="lg")
                    for kc in range(2):
                        nc.tensor.matmul(
                            ps[:, kc * 512:(kc + 1) * 512],
                            qT_h[:, mi * P:(mi + 1) * P],
                            kT_h[:, kc * 512:(kc + 1) * 512],
                            start=True,
                            stop=True,
                        )
                    exa = exA_pool.tile([P, NK], FP, tag="exA")
                    si = h * 8 + mi
                    nc.scalar.activation(
                        exa[:], ps[:], Exp, scale=SCALE,
                        accum_out=sums[:, si:si + 1],
                    )
                    exa_tiles.append(exa)

                nc.vector.reciprocal(rec[:, h * 8:(h + 1) * 8], sums[:, h * 8:(h + 1) * 8])

                # --- B: logitsT [k, q] -> expT (bf16) ---
                expT = expT_pool.tile([P, 8, NQ], BF, tag="expT")
                for kj in range(8):
                    ps = ps_lg.tile([P, NQ], FP, tag="lg")
                    for qc in range(2):
                        nc.tensor.matmul(
                            ps[:, qc * 512:(qc + 1) * 512],
                            kT_h[:, kj * P:(kj + 1) * P],
                            qT_h[:, qc * 512:(qc + 1) * 512],
                            start=True,
                            stop=True,
                        )
                    nc.scalar.activation(expT[:, kj], ps[:], Exp, scale=SCALE)

                # --- normalize attn in place + store ---
                for mi in range(8):
                    si = h * 8 + mi
                    nc.vector.tensor_scalar_mul(
                        exa_tiles[mi][:], exa_tiles[mi][:], rec[:, si:si + 1]
                    )
                    nc.sync.dma_start(
                        attn_o[h, mi * P:(mi + 1) * P, :], exa_tiles[mi][:]
                    )

                # --- AV: out_u[q, 64] = sum_kj expT_kj^T v_kj; scale by rec ---
                for mi in range(8):
                    psv = ps_av.tile([P, HD], FP, tag="av")
                    for kj in range(8):
                        nc.tensor.matmul(
                            psv[:],
                            expT[:, kj, mi * P:(mi + 1) * P],
                            v_sb[:, kj, h],
                            start=(kj == 0),
                            stop=(kj == 7),
                        )
                    si = h * 8 + mi
                    nc.vector.tensor_scalar_mul(
                        out_n[:, mi, h * HD:(h + 1) * HD], psv[:], rec[:, si:si + 1]
                    )

            # ---- transpose out [q, dh] -> outT [dh, q] ----
            outT = persist.tile([P, 2, NQ], FR)
            for mi in range(8):
                for j in range(2):
                    pst = ps_av.tile([P, P], FP, tag="av")
                    nc.tensor.transpose(pst[:], out_n[:, mi, j * P:(j + 1) * P], ident[:])
                    nc.vector.tensor_copy(outT[:, j, mi * P:(mi + 1) * P], pst[:])

            # ---- output projection ----
            for mi in range(8):
                ps = ps_mm.tile([P, D], FP, tag="mm")
                for j in range(2):
                    nc.tensor.matmul(
                        ps[:],
                        outT[:, j, mi * P:(mi + 1) * P],
                        wp_sb[:, j, :],
                        start=(j == 0),
                        stop=(j == 1),
                    )
                of = outp.tile([P, D], FP, tag="of")
                nc.vector.tensor_copy(of[:], ps[:])
                nc.sync.dma_start(out_o[mi * P:(mi + 1) * P, :], of[:])

    nc.compile()
    return nc


def get_nc():
    if "nc" not in _NC_CACHE:
        _NC_CACHE["nc"] = build_nc()
    return _NC_CACHE["nc"]


def make_in_maps(x, x_q, w_q, w_kv):
    """Shard full inputs into 8 per-core input maps (host-side numpy)."""
    x = np.asarray(x, dtype=np.float32)
    x_q = np.asarray(x_q, dtype=np.float32)
    w_q = np.asarray(w_q, dtype=np.float32)
    w_kv = np.asarray(w_kv, dtype=np.float32)
    in_maps = []
    for c in range(NCORES):
        b, hg = c // 2, c % 2
        sl = slice(hg * DH, (hg + 1) * DH)
        in_maps.append({
            "xqT": np.ascontiguousarray(x_q[b].T),
            "xT": np.ascontiguousarray(x[b].T),
            "wq": np.ascontiguousarray(w_q[:, sl]),
            "wk": np.ascontiguousarray(w_kv[:, sl]),
            "wv": np.ascontiguousarray(w_kv[:, D + hg * DH:D + (hg + 1) * DH]),
        })
    return in_maps


def make_in_maps_full(x, x_q, w_q, w_kv, w_proj):
    w_proj = np.asarray(w_proj, dtype=np.float32)
    in_maps = make_in_maps(x, x_q, w_q, w_kv)
    for c in range(NCORES):
        hg = c % 2
        sl = slice(hg * DH, (hg + 1) * DH)
        in_maps[c]["wp"] = np.ascontiguousarray(w_proj[sl, :])
    return in_maps


def unshard(results, b_proj):
    b_proj = np.asarray(b_proj, dtype=np.float32)
    attn = np.empty((B, H, NQ, NK), dtype=np.float32)
    out = np.empty((B, NQ, D), dtype=np.float32)
    for c in range(NCORES):
        b, hg = c // 2, c % 2
        attn[b, hg * HPC:(hg + 1) * HPC] = results[c]["attn_o"]
    for b in range(B):
        out[b] = results[2 * b]["out_o"] + results[2 * b + 1]["out_o"] + b_proj[None, :]
    return out, attn


def kernel(x, x_q, w_q, w_kv, w_proj, b_proj):
    from concourse.bass_utils import run_bass_kernel_spmd

    nc = get_nc()
    in_maps = make_in_maps_full(x, x_q, w_q, w_kv, w_proj)
    res = run_bass_kernel_spmd(nc, in_maps, list(range(NCORES))).results
    return unshard(res, b_proj)


# revision 10
# speedup vs baseline: 3.7292x; 1.0698x over previous
"""Trainium2 Bass kernel for nn_Attention (B=4, Nq=Nk=1024, D=512, H=8).

Sharding: 8 cores = 4 batches x 2 head-groups (4 heads each).
Core c handles batch b = c // 2, heads [hg*4, hg*4+4) with hg = c % 2.

Per-core device program (all heavy math on device):
  qT = (x_q[b] @ w_q[:, hg])^T          [256, 1024]   (f32r matmuls)
  kT = (x[b]  @ w_k[:, hg])^T           [256, 1024]
  v  =  x[b]  @ w_v[:, hg]              [1024, 256]   (stored bf16)
  per head h (4):
    A: logits[q,k] = qT_h^T kT_h; exp (ACT, scale=0.125 fused, accum row
       sums); normalize in-place (gpsimd, per-partition recip); DMA -> attn
    B: logitsT[k,q] = kT_h^T qT_h; expT = exp (ACT, bf16)
    AV: out_u[q,64] = sum_kj expT_kj^T v_kj (bf16 matmuls);
        scale by recip (DVE, fused into PSUM->SBUF copy)
  transpose out [1024,256] -> outT (PE transposes), proj = outT^T @ w_p
  partial out -> DRAM (host adds the two head-group partials + bias).

Host side: slice/transpose inputs (numpy), run SPMD on 8 cores, concat
attn shards, add the two per-batch partials + b_proj.
"""

import sys

import numpy as np

for _p in ("/opt/trn_rl_repo",):
    if _p not in sys.path:
        sys.path.insert(0, _p)

# Problem constants (hardcoded per contest rules).
B, NQ, NK = 4, 1024, 1024
D = 512          # DIM_Q = DIM_K = OUT_DIM
H = 8
HD = 64          # head dim
SCALE = HD ** -0.5
HPC = 4          # heads per core
DH = HPC * HD    # 256: per-core slice of head dims
P = 128
KO = D // P      # 4 contraction chunks for the input projections
NCORES = 8

_NC_CACHE = {}


def build_nc():
    """Build the (single) SPMD Bass program. Same program on all 8 cores."""
    from contextlib import ExitStack

    import concourse.mybir as mybir
    import concourse.tile as tile
    from concourse import bacc
    from concourse.masks import make_identity

    FP = mybir.dt.float32
    BF = mybir.dt.bfloat16
    FR = mybir.dt.float32r
    Exp = mybir.ActivationFunctionType.Exp

    nc = bacc.Bacc("TRN2")
    xqT = nc.declare_dram_parameter("xqT", [D, NQ], FR, isOutput=False)
    xT = nc.declare_dram_parameter("xT", [D, NK], FR, isOutput=False)
    wq = nc.declare_dram_parameter("wq", [D, DH], FR, isOutput=False)
    wk = nc.declare_dram_parameter("wk", [D, DH], FR, isOutput=False)
    wv = nc.declare_dram_parameter("wv", [D, DH], FR, isOutput=False)
    wp = nc.declare_dram_parameter("wp", [DH, D], BF, isOutput=False)
    attn_o = nc.declare_dram_parameter("attn_o", [HPC, NQ, NK], FP, isOutput=True)
    out_o = nc.declare_dram_parameter("out_o", [NQ, D], FP, isOutput=True)

    with tile.TileContext(nc) as tc:
        with ExitStack() as ctx:
            consts = ctx.enter_context(tc.tile_pool(name="consts", bufs=1))
            persist = ctx.enter_context(tc.tile_pool(name="persist", bufs=1))
            expT_pool = ctx.enter_context(tc.tile_pool(name="expT", bufs=2))
            exA_pool = ctx.enter_context(tc.tile_pool(name="exA", bufs=10))
            outp = ctx.enter_context(tc.tile_pool(name="outp", bufs=2))
            ps_mm = ctx.enter_context(tc.tile_pool(name="ps_mm", bufs=2, space="PSUM"))
            ps_lg = ctx.enter_context(tc.tile_pool(name="ps_lg", bufs=2, space="PSUM"))
            ps_av = ctx.enter_context(tc.tile_pool(name="ps_av", bufs=2, space="PSUM"))

            # ---- load inputs ----
            xqT_sb = persist.tile([P, KO, NQ], FR)
            nc.sync.dma_start(xqT_sb[:], xqT[:].rearrange("(ko p) q -> p ko q", p=P))
            xT_sb = persist.tile([P, KO, NK], FR)
            nc.sync.dma_start(xT_sb[:], xT[:].rearrange("(ko p) q -> p ko q", p=P))
            wq_sb = persist.tile([P, KO, DH], FR)
            nc.sync.dma_start(wq_sb[:], wq[:].rearrange("(ko p) m -> p ko m", p=P))
            wk_sb = persist.tile([P, KO, DH], FR)
            nc.sync.dma_start(wk_sb[:], wk[:].rearrange("(ko p) m -> p ko m", p=P))
            wv_sb = persist.tile([P, KO, DH], FR)
            nc.sync.dma_start(wv_sb[:], wv[:].rearrange("(ko p) m -> p ko m", p=P))
            wp_sb = persist.tile([P, 2, D], BF)
            nc.sync.dma_start(wp_sb[:], wp[:].rearrange("(j p) n -> p j n", p=P))
            ident = consts.tile([P, P], BF)
            make_identity(nc, ident[:])

            # ---- projections: qT, kT  ([dh-part(2x128), seq]) ----
            qT_sb = persist.tile([P, 2, NQ], BF)
            kT_sb = persist.tile([P, 2, NK], BF)
            for w_sb, x_sb, dst in ((wq_sb, xqT_sb, qT_sb), (wk_sb, xT_sb, kT_sb)):
                for j in range(2):
                    for qc in range(2):
                        ps = ps_mm.tile([P, 512], FP, tag="mm")
                        for ko in range(KO):
                            nc.tensor.matmul(
                                ps[:],
                                w_sb[:, ko, j * P:(j + 1) * P],
                                x_sb[:, ko, qc * 512:(qc + 1) * 512],
                                start=(ko == 0),
                                stop=(ko == KO - 1),
                            )
                        nc.vector.tensor_copy(dst[:, j, qc * 512:(qc + 1) * 512], ps[:])

            # ---- v in natural layout [k-part, kj, head, hd], bf16 ----
            v_sb = persist.tile([P, 8, HPC, HD], BF)
            for kj in range(8):
                ps = ps_mm.tile([P, DH], FP, tag="mm")
                for ko in range(KO):
                    nc.tensor.matmul(
                        ps[:],
                        xT_sb[:, ko, kj * P:(kj + 1) * P],
                        wv_sb[:, ko, :],
                        start=(ko == 0),
                        stop=(ko == KO - 1),
                    )
                nc.vector.tensor_copy(
                    v_sb[:, kj].rearrange("p h d -> p (h d)"), ps[:]
                )

            # ---- per-head attention ----
            sums = consts.tile([P, HPC * 8], FP)   # row sums per (head, q-tile)
            rec = consts.tile([P, HPC * 8], FP)    # reciprocals
            out_n = persist.tile([P, 8, DH], BF)   # normalized attn @ v, [q, dh]

            for h in range(HPC):
                j, p0 = h // 2, (h % 2) * 64
                qT_h = qT_sb[p0:p0 + 64, j]        # [64, NQ]
                kT_h = kT_sb[p0:p0 + 64, j]        # [64, NK]

                # --- A: logits [q, k] -> exp (+ row sums) ---
                exa_tiles = []
                for mi in range(8):
                    ps = ps_lg.tile([P, NK], FP, tag="lg")
                    for kc in range(2):
                        nc.tensor.matmul(
                            ps[:, kc * 512:(kc + 1) * 512],
                            qT_h[:, mi * P:(mi + 1) * P],
                            kT_h[:, kc * 512:(kc + 1) * 512],
                            start=True,
                            stop=True,
                        )
                    exa = exA_pool.tile([P, NK], FP, tag="exA")
                    si = h * 8 + mi
                    nc.scalar.activation(
                        exa[:], ps[:], Exp, scale=SCALE,
                        accum_out=sums[:, si:si + 1],
                    )
                    exa_tiles.append(exa)

                nc.vector.reciprocal(rec[:, h * 8:(h + 1) * 8], sums[:, h * 8:(h + 1) * 8])

                # --- B: logitsT [k, q] -> expT (bf16) ---
                expT = expT_pool.tile([P, 8, NQ], BF, tag="expT")
                for kj in range(8):
                    ps = ps_lg.tile([P, NQ], FP, tag="lg")
                    for qc in range(2):
                        nc.tensor.matmul(
                            ps[:, qc * 512:(qc + 1) * 512],
                            kT_h[:, kj * P:(kj + 1) * P],
                            qT_h[:, qc * 512:(qc + 1) * 512],
                            start=True,
                            stop=True,
                        )
                    nc.scalar.activation(expT[:, kj], ps[:], Exp, scale=SCALE)

                # --- normalize attn in place + store ---
                for mi in range(8):
                    si = h * 8 + mi
                    nc.vector.tensor_scalar_mul(
                        exa_tiles[mi][:], exa_tiles[mi][:], rec[:, si:si + 1]
                    )
                    nc.sync.dma_start(
                        attn_o[h, mi * P:(mi + 1) * P, :], exa_tiles[mi][:]
                    )

                # --- AV: out_u[q, 64] = sum_kj expT_kj^T v_kj; scale by rec ---
                for mi in range(8):
                    psv = ps_av.tile([P, HD], FP, tag="av")
                    for kj in range(8):
                        nc.tensor.matmul(
                            psv[:],
                            expT[:, kj, mi * P:(mi + 1) * P],
                            v_sb[:, kj, h],
                            start=(kj == 0),
                            stop=(kj == 7),
                        )
                    si = h * 8 + mi
                    nc.vector.tensor_scalar_mul(
                        out_n[:, mi, h * HD:(h + 1) * HD], psv[:], rec[:, si:si + 1]
                    )

            # ---- transpose out [q, dh] -> outT [dh, q] ----
            outT = persist.tile([P, 2, NQ], BF)
            for mi in range(8):
                for j in range(2):
                    pst = ps_av.tile([P, P], BF, tag="av")
                    nc.tensor.transpose(pst[:], out_n[:, mi, j * P:(j + 1) * P], ident[:])
                    nc.vector.tensor_copy(outT[:, j, mi * P:(mi + 1) * P], pst[:])

            # ---- output projection ----
            for mi in range(8):
                ps = ps_mm.tile([P, D], FP, tag="mm")
                for j in range(2):
                    nc.tensor.matmul(
                        ps[:],
                        outT[:, j, mi * P:(mi + 1) * P],
                        wp_sb[:, j, :],
                        start=(j == 0),
                        stop=(j == 1),
                    )
                of = outp.tile([P, D], FP, tag="of")
                nc.vector.tensor_copy(of[:], ps[:])
                nc.sync.dma_start(out_o[mi * P:(mi + 1) * P, :], of[:])

    nc.compile()
    return nc


def get_nc():
    if "nc" not in _NC_CACHE:
        _NC_CACHE["nc"] = build_nc()
    return _NC_CACHE["nc"]


def make_in_maps(x, x_q, w_q, w_kv):
    """Shard full inputs into 8 per-core input maps (host-side numpy)."""
    x = np.asarray(x, dtype=np.float32)
    x_q = np.asarray(x_q, dtype=np.float32)
    w_q = np.asarray(w_q, dtype=np.float32)
    w_kv = np.asarray(w_kv, dtype=np.float32)
    in_maps = []
    for c in range(NCORES):
        b, hg = c // 2, c % 2
        sl = slice(hg * DH, (hg + 1) * DH)
        in_maps.append({
            "xqT": np.ascontiguousarray(x_q[b].T),
            "xT": np.ascontiguousarray(x[b].T),
            "wq": np.ascontiguousarray(w_q[:, sl]),
            "wk": np.ascontiguousarray(w_kv[:, sl]),
            "wv": np.ascontiguousarray(w_kv[:, D + hg * DH:D + (hg + 1) * DH]),
        })
    return in_maps


def make_in_maps_full(x, x_q, w_q, w_kv, w_proj):
    import ml_dtypes

    w_proj = np.asarray(w_proj, dtype=np.float32)
    in_maps = make_in_maps(x, x_q, w_q, w_kv)
    for c in range(NCORES):
        hg = c % 2
        sl = slice(hg * DH, (hg + 1) * DH)
        in_maps[c]["wp"] = np.ascontiguousarray(
            w_proj[sl, :].astype(ml_dtypes.bfloat16)
        )
    return in_maps


def unshard(results, b_proj):
    b_proj = np.asarray(b_proj, dtype=np.float32)
    attn = np.empty((B, H, NQ, NK), dtype=np.float32)
    out = np.empty((B, NQ, D), dtype=np.float32)
    for c in range(NCORES):
        b, hg = c // 2, c % 2
        attn[b, hg * HPC:(hg + 1) * HPC] = results[c]["attn_o"]
    for b in range(B):
        out[b] = results[2 * b]["out_o"] + results[2 * b + 1]["out_o"] + b_proj[None, :]
    return out, attn


def kernel(x, x_q, w_q, w_kv, w_proj, b_proj):
    from concourse.bass_utils import run_bass_kernel_spmd

    nc = get_nc()
    in_maps = make_in_maps_full(x, x_q, w_q, w_kv, w_proj)
    res = run_bass_kernel_spmd(nc, in_maps, list(range(NCORES))).results
    return unshard(res, b_proj)
